# revision 1
# baseline (speedup 1.0000x reference)
"""Trainium2 Bass kernel for nn_Attention (dense transformer block:
qkv proj + RoPE + causal attention + out proj), tensor-parallel over
8 NeuronCores: core c handles batch b=c//2, head-group g=c%2 (8 heads).

Self-contained: hardcodes all shapes; host preps transposed/permuted
shards, device computes partial y per core, host sums head-group pairs
and adds the output bias.
"""

from contextlib import ExitStack

import numpy as np

import concourse.bass as bass
import concourse.tile as tile
from concourse import bacc, mybir
from concourse.bass import ds, ts
from concourse.bass_utils import run_bass_kernel_spmd

B, S, D, H, DH = 4, 2048, 1024, 16, 64
HL = 8          # heads per core
INNER = H * DH  # 1024
KC = D // 128   # 8 contraction chunks
NT = S // 128   # 16 token tiles
F32 = mybir.dt.float32
F32R = mybir.dt.float32r
MM_FP32R = True  # float32r matmuls: 1 cyc/row at N>=256 (vs 4 for fp32)




def _pieces(cw):
    """split a psum-tile column span into single-bank matmul pieces"""
    out = [(i * 512, 512) for i in range(cw // 512)]
    if cw % 512:
        out.append((cw - cw % 512, cw % 512))
    return out


def build_kernel(nc, phases=3):
    xT = nc.dram_tensor("xT", [D, S], F32R, kind="ExternalInput").ap()
    wq = nc.dram_tensor("wq", [D, HL * DH], F32R, kind="ExternalInput").ap()
    wk = nc.dram_tensor("wk", [D, HL * DH], F32R, kind="ExternalInput").ap()
    wv = nc.dram_tensor("wv", [D, HL * DH], F32R, kind="ExternalInput").ap()
    wo = nc.dram_tensor("wo", [HL * DH, D], F32R, kind="ExternalInput").ap()
    cc = nc.dram_tensor("cc", [128, S], F32, kind="ExternalInput").ap()
    ssw = nc.dram_tensor("ssw", [128, S], F32, kind="ExternalInput").ap()
    y = nc.dram_tensor("y", [S, D], F32, kind="ExternalOutput").ap()

    EXP = mybir.ActivationFunctionType.Exp
    SCALE = 1.0 / np.sqrt(DH)

    with tile.TileContext(nc) as tc, ExitStack() as top:
        opool = top.enter_context(tc.tile_pool(name="opool", bufs=1))
        ot = [None] * 4

        with ExitStack() as mid:
            qkp = mid.enter_context(tc.tile_pool(name="qkt", bufs=1))
            vpool = mid.enter_context(tc.tile_pool(name="vpool", bufs=1))
            qkt = [qkp.tile([128, S], F32R, tag=f"qkt{t}", name=f"qkt{t}") for t in range(8)]
            vsb = vpool.tile([128, NT, HL, DH + 1], F32R, tag="vsb", name="vsb")

            # ---------------- phase B: projections + rope -----------------
            with ExitStack() as ph:
                consts = ph.enter_context(tc.tile_pool(name="consts", bufs=1))
                xtp = ph.enter_context(tc.tile_pool(name="xtp", bufs=1))
                wsl = ph.enter_context(tc.tile_pool(name="wsl", bufs=2))
                rtmp = ph.enter_context(tc.tile_pool(name="rtmp", bufs=3))
                psqk = ph.enter_context(
                    tc.tile_pool(name="psqk", bufs=2, space="PSUM"))
                psv = ph.enter_context(
                    tc.tile_pool(name="psv", bufs=2, space="PSUM"))

                wv_sb = consts.tile([128, KC, 512], F32R, tag="wv", name="wv")
                nc.sync.dma_start(
                    wv_sb[:], wv.rearrange("(k p) n -> p k n", p=128))

                nc.gpsimd.memset(vsb[:, :, :, DH].bitcast(F32), 1.0)

                for half in range(2):
                    hs = ds(half * 1024, 1024)
                    cc_sb = consts.tile([128, 1024], F32, tag="cch", name="cch")
                    nc.sync.dma_start(cc_sb[:], cc[:, hs])
                    ssw_sb = consts.tile([128, 1024], F32, tag="sswh", name="sswh")
                    nc.sync.dma_start(ssw_sb[:], ssw[:, hs])
                    xth = []
                    for k in range(KC):
                        xh = xtp.tile([128, 1024], F32R, tag=f"xth{k}", name=f"xth{k}")
                        nc.sync.dma_start(
                            xh[:], xT[ts(k, 128), ds(half * 1024, 1024)])
                        xth.append(xh)
                    # q/k projections interleaved with v projection
                    for t in range(8):
                        wsrc = wq if t < 4 else wk
                        m = t % 4
                        wt8 = [wsl.tile([128, 4, 128], F32R, tag=f"w{i}", name=f"w{i}")
                               for i in range(2)]
                        for i in range(2):
                            nc.sync.dma_start(
                                wt8[i][:],
                                wsrc.rearrange("(g k p) n -> g p k n", g=2, p=128)[i][:, :, ts(m, 128)])
                        ps = psqk.tile([128, 1024], F32, tag="psqk")
                        for k in range(KC):
                            for p2 in range(2):
                                nc.tensor.matmul(
                                    ps[:, ts(p2, 512)],
                                    (wt8[k // 4][:, k % 4, :]),
                                    (xth[k][:, ts(p2, 512)]),
                                    start=(k == 0), stop=(k == KC - 1))
                        # rope: out = t*CC + swap32(t*SSsw)
                        nc.vector.tensor_mul(qkt[t][:, hs], ps[:], cc_sb[:])
                        v2 = rtmp.tile([128, 1024], F32, tag="v2")
                        nc.vector.tensor_mul(v2[:], ps[:], ssw_sb[:])
                        v2s = rtmp.tile([128, 1024], F32, tag="v2", name="v2s")
                        for blk in range(4):
                            src = (blk ^ 1) * 32
                            nc.scalar.dma_start(
                                v2s[ds(blk * 32, 32), :], v2[ds(src, 32), :])
                        nc.gpsimd.tensor_tensor(
                            qkt[t][:, hs], qkt[t][:, hs], v2s[:],
                            op=mybir.AluOpType.add)
                        # v projection tile for this slot
                        tt = half * 8 + t
                        psV = psv.tile([128, 512], F32, tag="psv")
                        for k in range(KC):
                            nc.tensor.matmul(
                                psV[:], (xth[k][:, ds(t * 128, 128)]),
                                (wv_sb[:, k, :]),
                                start=(k == 0), stop=(k == KC - 1))
                        nc.scalar.copy(
                            vsb[:, tt, :, 0:DH],
                            psV[:].rearrange("p (h d) -> p h d", h=HL))

            # ---------------- attention ----------------------------------
            if phases < 2:
                return nc
            with ExitStack() as ph:
                ppool = ph.enter_context(tc.tile_pool(name="ppool", bufs=5))
                lpool = ph.enter_context(tc.tile_pool(name="lpool", bufs=2))
                pssc = ph.enter_context(
                    tc.tile_pool(name="pssc", bufs=2, space="PSUM"))
                psav = ph.enter_context(
                    tc.tile_pool(name="psav", bufs=2, space="PSUM"))

                for h in range(HL):
                    ht, hb = h // 2, 64 * (h % 2)
                    if ot[ht] is None:
                        ot[ht] = opool.tile([128, S], F32R, tag=f"ot{ht}", name=f"ot{ht}")
                    q_ap = qkt[ht][ds(hb, 64), :]
                    k_ap = qkt[4 + ht][ds(hb, 64), :]
                    for qh in range(2):
                        q0, q1 = 1024 * qh, 1024 * (qh + 1)
                        pav = psav.tile([DH + 1, 1024], F32, tag="pav")
                        for j in range(8 * (qh + 1)):
                            gs = max(q0, 128 * j)     # first valid q col
                            cw = q1 - gs
                            ps = pssc.tile([128, cw], F32, tag="sc")
                            for (po, pw) in _pieces(cw):
                                nc.tensor.matmul(
                                    ps[:, ds(po, pw)],
                                    (k_ap[:, ds(128 * j, 128)]),
                                    (q_ap[:, ds(gs + po, pw)]),
                                    start=True, stop=True)
                            pj = ppool.tile([128, cw], F32R, tag="P")
                            nc.scalar.activation(pj[:], ps[:], EXP, scale=SCALE)
                            if gs == 128 * j:
                                # diagonal block: causal-mask first 128 cols
                                nc.gpsimd.affine_select(
                                    out=pj[:, 0:128], in_=pj[:, 0:128],
                                    compare_op=mybir.AluOpType.is_ge, fill=0.0,
                                    base=0, pattern=[[1, 128]],
                                    channel_multiplier=-1)
                            for c in range(max(2 * qh, j // 4), 2 * qh + 2):
                                cs = max(512 * c, 128 * j)
                                w = 512 * (c + 1) - cs
                                nc.tensor.matmul(
                                    pav[:, ds(cs - q0, w)],
                                    (vsb[:, j, h, :]),
                                    (pj[:, ds(cs - gs, w)]),
                                    start=(j == 0),
                                    stop=(j == min(8 * (qh + 1) - 1, 4 * c + 3)))
                        # normalize: ot rows = pav[:64] / l, l = pav[64]
                        qsl = ds(q0, 1024)
                        lr = lpool.tile([128, 1024], F32, tag="lr")
                        nc.vector.tensor_copy(lr[ds(64, 1), :], pav[ds(DH, 1), :])
                        nc.sync.dma_start(lr[ds(0, 1), :], lr[ds(64, 1), :])
                        nc.vector.reciprocal(lr[ds(0, 1), :], lr[ds(0, 1), :])
                        rb = lpool.tile([64, 1024], F32, tag="rb")
                        nc.gpsimd.partition_broadcast(rb[:], lr[ds(0, 1), :],
                                                      channels=64)
                        if h % 2 == 0:
                            nc.vector.tensor_mul(
                                ot[ht][ds(0, 64), qsl], pav[ds(0, DH), :], rb[:])
                        else:
                            ott = lpool.tile([64, 1024], F32R, tag="ott")
                            nc.vector.tensor_mul(ott[:], pav[ds(0, DH), :], rb[:])
                            nc.sync.dma_start(ot[ht][ds(64, 64), qsl], ott[:])

        # ---------------- out projection ---------------------------------
        if phases < 3:
            return nc
        with ExitStack() as ph:
            wop = ph.enter_context(tc.tile_pool(name="wop", bufs=1))
            ypool = ph.enter_context(tc.tile_pool(name="ypool", bufs=3))
            psy = ph.enter_context(
                tc.tile_pool(name="psy", bufs=2, space="PSUM"))
            wo_sb = [wop.tile([128, D], F32R, tag=f"wo{k}", name=f"wo{k}") for k in range(4)]
            for k in range(4):
                nc.sync.dma_start(wo_sb[k][:], wo[ts(k, 128), :])
            for tt in range(NT):
                ps = psy.tile([128, D], F32, tag="psy")
                for k in range(4):
                    for half in range(2):
                        nc.tensor.matmul(
                            ps[:, ts(half, 512)],
                            (ot[k][:, ts(tt, 128)]),
                            (wo_sb[k][:, ts(half, 512)]),
                            start=(k == 0), stop=(k == 3))
                ysb = ypool.tile([128, D], F32, tag="y")
                nc.vector.tensor_copy(ysb[:], ps[:])
                nc.sync.dma_start(y[ts(tt, 128), :], ysb[:])
    return nc


# ---------------- host side ------------------------------------------------

def _rope_tables():
    i = np.arange(DH // 2, dtype=np.float32)
    thetas = np.power(np.float32(10000.0), -2.0 * (i - 1.0) / DH)
    vals = thetas[:, None].astype(np.float32) * \
        np.arange(S, dtype=np.float32)[None, :]
    cos32 = np.cos(vals).astype(np.float32)
    sin32 = np.sin(vals).astype(np.float32)
    CC = np.tile(cos32, (4, 1))
    SSsw = np.concatenate([sin32, -sin32, sin32, -sin32], axis=0)
    return np.ascontiguousarray(CC), np.ascontiguousarray(SSsw)


def _qk_col_perm(g):
    cols = []
    for m in range(4):
        for hh in (2 * m, 2 * m + 1):
            hg = HL * g + hh
            cols += [hg * DH + 2 * i for i in range(32)]
            cols += [hg * DH + 2 * i + 1 for i in range(32)]
    return np.array(cols)


_CACHE = {}


def _get_module():
    if "nc" not in _CACHE:
        nc = bacc.Bacc("TRN2", target_bir_lowering=False, debug=False,
                       num_devices=8)
        build_kernel(nc)
        nc.compile()
        _CACHE["nc"] = nc
    return _CACHE["nc"]


def make_in_maps(x, Wqkv, Wout):
    x = np.ascontiguousarray(np.asarray(x, np.float32))
    Wqkv = np.ascontiguousarray(np.asarray(Wqkv, np.float32))
    Wout = np.ascontiguousarray(np.asarray(Wout, np.float32))
    CC, SSsw = _rope_tables()
    shard = {}
    for g in range(2):
        perm = _qk_col_perm(g)
        vcols = np.arange(HL * g * DH, HL * (g + 1) * DH)
        shard[g] = dict(
            wq=np.ascontiguousarray(Wqkv[:, 0 * INNER:1 * INNER][:, perm]),
            wk=np.ascontiguousarray(Wqkv[:, 1 * INNER:2 * INNER][:, perm]),
            wv=np.ascontiguousarray(Wqkv[:, 2 * INNER:3 * INNER][:, vcols]),
            wo=np.ascontiguousarray(Wout[vcols, :]),
        )
    in_maps = []
    for c in range(8):
        b, g = c // 2, c % 2
        in_maps.append(dict(
            xT=np.ascontiguousarray(x[b].T), cc=CC, ssw=SSsw, **shard[g]))
    return in_maps


def kernel(x, Wqkv, Wout, bout):
    bout = np.asarray(bout, np.float32)
    nc = _get_module()
    in_maps = make_in_maps(x, Wqkv, Wout)
    res = run_bass_kernel_spmd(nc, in_maps, core_ids=list(range(8)))
    ys = [r["y"] for r in res.results]
    out = np.stack([ys[2 * b] + ys[2 * b + 1] + bout for b in range(B)])
    return out.astype(np.float32)



# revision 17
# speedup vs baseline: 82.6669x; 82.6669x over previous
"""Trainium2 Bass kernel for nn_Attention (dense transformer block:
qkv proj + RoPE + causal attention + out proj), tensor-parallel over
8 NeuronCores: core c handles batch b=c//2, head-group g=c%2 (8 heads).

Self-contained: hardcodes all shapes; host preps transposed/permuted
bf16 shards, device computes partial y per core, host sums head-group
pairs and adds the output bias.

v2: bf16 matmul operands (fp32 psum accumulation), softmax 1/l via
ACT exp(-ln(l)) instead of DVE reciprocal, and attention/out-proj
interleaved with the projection stream so the PE never idles long
enough for the HAM clock gate to re-throttle.
"""

from contextlib import ExitStack

import numpy as np
import ml_dtypes

import concourse.bass as bass
import concourse.tile as tile
from concourse import bacc, mybir
from concourse.bass import ds, ts
from concourse.bass_utils import run_bass_kernel_spmd

B, S, D, H, DH = 4, 2048, 1024, 16, 64
HL = 8          # heads per core
INNER = H * DH  # 1024
KC = D // 128   # 8 contraction chunks
NT = S // 128   # 16 token tiles
F32 = mybir.dt.float32
BF16 = mybir.dt.bfloat16

EXP = mybir.ActivationFunctionType.Exp
LN = mybir.ActivationFunctionType.Ln
SCALE = 1.0 / np.sqrt(DH)


def build_kernel(nc):
    xT = nc.dram_tensor("xT", [D, S], BF16, kind="ExternalInput").ap()
    wq = nc.dram_tensor("wq", [D, HL * DH], BF16, kind="ExternalInput").ap()
    wk = nc.dram_tensor("wk", [D, HL * DH], BF16, kind="ExternalInput").ap()
    wv = nc.dram_tensor("wv", [D, HL * DH], BF16, kind="ExternalInput").ap()
    wo = nc.dram_tensor("wo", [HL * DH, D], BF16, kind="ExternalInput").ap()
    cc = nc.dram_tensor("cc", [128, S], F32, kind="ExternalInput").ap()
    ssw = nc.dram_tensor("ssw", [128, S], F32, kind="ExternalInput").ap()
    y = nc.dram_tensor("y", [S, D], F32, kind="ExternalOutput").ap()

    with tile.TileContext(nc) as tc, ExitStack() as top:
        consts = top.enter_context(tc.tile_pool(name="consts", bufs=1))
        xtp = top.enter_context(tc.tile_pool(name="xtp", bufs=1))
        qkp = top.enter_context(tc.tile_pool(name="qkp", bufs=1))
        vpool = top.enter_context(tc.tile_pool(name="vpool", bufs=1))
        opool = top.enter_context(tc.tile_pool(name="opool", bufs=1))
        rtmp = top.enter_context(tc.tile_pool(name="rtmp", bufs=2))
        ppool = top.enter_context(tc.tile_pool(name="ppool", bufs=3))
        apool = top.enter_context(tc.tile_pool(name="apool", bufs=1))
        lpool = top.enter_context(tc.tile_pool(name="lpool", bufs=1))
        ypool = top.enter_context(tc.tile_pool(name="ypool", bufs=3))
        ps512 = top.enter_context(
            tc.tile_pool(name="ps512", bufs=2, space="PSUM"))
        pssc = top.enter_context(
            tc.tile_pool(name="pssc", bufs=2, space="PSUM"))
        psav = top.enter_context(
            tc.tile_pool(name="psav", bufs=1, space="PSUM"))

        # ---------------- constant / input loads --------------------------
        # order matters: wv + x-half0 first (v_proj starts the kernel),
        # wq/wk + rope tables next, wo (needed last) at the end
        wv_sb = consts.tile([128, KC, 512], BF16, tag="wv", name="wv")
        nc.sync.dma_start(wv_sb[:], wv.rearrange("(k p) n -> p k n", p=128))
        xth = [xtp.tile([128, S], BF16, tag=f"xth{k}", name=f"xth{k}")
               for k in range(KC)]
        for k in range(KC):
            nc.sync.dma_start(xth[k][:, 0:1024], xT[ts(k, 128), 0:1024])
        wq_sb = consts.tile([128, KC, 512], BF16, tag="wq", name="wq")
        nc.sync.dma_start(wq_sb[:], wq.rearrange("(k p) n -> p k n", p=128))
        wk_sb = consts.tile([128, KC, 512], BF16, tag="wk", name="wk")
        nc.sync.dma_start(wk_sb[:], wk.rearrange("(k p) n -> p k n", p=128))
        cc_sb = consts.tile([128, S], F32, tag="cc", name="cc")
        nc.sync.dma_start(cc_sb[:], cc[:])
        ssw_sb = consts.tile([128, S], F32, tag="ssw", name="ssw")
        nc.sync.dma_start(ssw_sb[:], ssw[:])
        for k in range(KC):
            nc.sync.dma_start(xth[k][:, 1024:2048], xT[ts(k, 128), 1024:2048])
        wo_sb = [consts.tile([128, D], BF16, tag=f"wo{k}", name=f"wo{k}")
                 for k in range(4)]
        for k in range(4):
            nc.sync.dma_start(wo_sb[k][:], wo[ts(k, 128), :])

        qkt = [qkp.tile([128, S], BF16, tag=f"qkt{t}", name=f"qkt{t}")
               for t in range(8)]
        vsb = vpool.tile([128, NT, HL, DH + 1], BF16, tag="vsb", name="vsb")
        nc.gpsimd.memset(vsb[:, :, :, DH], 1.0)
        ot = [opool.tile([128, S], BF16, tag=f"ot{m}", name=f"ot{m}")
              for m in range(4)]
        # constant strictly-upper-triangular zero mask for diagonal blocks
        trimask = consts.tile([128, 128], BF16, tag="trimask", name="trimask")
        nc.gpsimd.memset(trimask[:], 1.0)
        nc.gpsimd.affine_select(
            out=trimask[:], in_=trimask[:],
            compare_op=mybir.AluOpType.is_ge, fill=0.0,
            base=0, pattern=[[1, 128]], channel_multiplier=-1)

        # ---------------- building blocks ---------------------------------
        def v_proj(tt):
            half, tsub = divmod(tt, 8)
            psV = ps512.tile([128, 512], F32, tag="p512")
            for k in range(KC):
                nc.tensor.matmul(
                    psV[:], xth[k][:, ds(tt * 128, 128)], wv_sb[:, k, :],
                    start=(k == 0), stop=(k == KC - 1))
            nc.vector.tensor_copy(
                vsb[:, tt, :, 0:DH],
                psV[:].rearrange("p (h d) -> p h d", h=HL))

        def qk_proj(t, half):
            # q (t<4) / k (t>=4) projection + rope for 2 heads, one S-half
            wsrc = wq_sb if t < 4 else wk_sb
            m = t % 4
            for piece in range(2):
                csl = ds(half * 1024 + piece * 512, 512)
                ps = ps512.tile([128, 512], F32, tag="p512")
                for k in range(KC):
                    nc.tensor.matmul(
                        ps[:], wsrc[:, k, ts(m, 128)], xth[k][:, csl],
                        start=(k == 0), stop=(k == KC - 1))
                # rope: qkt = ps*CC + swap32(ps*SSsw)
                tmp = rtmp.tile([128, 512], BF16, tag="tmp")
                nc.vector.tensor_mul(tmp[:], ps[:], cc_sb[:, csl])
                v2 = rtmp.tile([128, 512], BF16, tag="v2")
                nc.vector.tensor_mul(v2[:], ps[:], ssw_sb[:, csl])
                v2s = rtmp.tile([128, 512], BF16, tag="v2s")
                for blk in range(4):
                    nc.scalar.dma_start(
                        v2s[ds(blk * 32, 32), :], v2[ds((blk ^ 1) * 32, 32), :])
                nc.gpsimd.tensor_tensor(
                    qkt[t][:, csl], tmp[:], v2s[:], op=mybir.AluOpType.add)

        def attention(hp, qh, avsb):
            # heads (2hp, 2hp+1); query tokens [qh*1024, (qh+1)*1024)
            q0 = qh * 1024
            for qt in range(2):
                qt0 = q0 + qt * 512
                jmax = (qt0 + 512) // 128 - 1
                pavs = [psav.tile([DH + 1, 512], F32, tag=f"pav{head}",
                                  name=f"pav{head}")
                        for head in range(2)]
                for j in range(jmax + 1):
                    gs = max(qt0, 128 * j)
                    w = qt0 + 512 - gs
                    # both heads' scores share one 2-bank psum tile (head1 in
                    # bank 1 at offset 512) so one exp can cover both heads
                    sc = pssc.tile([128, 1024], F32, tag="sc")
                    for head in range(2):
                        nc.tensor.matmul(
                            sc[:, ds(head * 512, w)],
                            qkt[4 + hp][ds(64 * head, 64), ds(128 * j, 128)],
                            qkt[hp][ds(64 * head, 64), ds(gs, w)],
                            start=True, stop=True)
                    pj = ppool.tile([128, 1024], BF16, tag="pj")
                    if w == 512:
                        nc.scalar.activation(pj[:], sc[:], EXP, scale=SCALE)
                    else:
                        for head in range(2):
                            nc.scalar.activation(
                                pj[:, ds(head * 512, w)],
                                sc[:, ds(head * 512, w)], EXP, scale=SCALE)
                    if gs == 128 * j:
                        # diagonal block: causal-mask first 128 cols per head
                        for head in range(2):
                            nc.vector.tensor_mul(
                                pj[:, ds(head * 512, 128)],
                                pj[:, ds(head * 512, 128)], trimask[:])
                    for head in range(2):
                        nc.tensor.matmul(
                            pavs[head][:, ds(gs - qt0, w)],
                            vsb[:, j, 2 * hp + head, :],
                            pj[:, ds(head * 512, w)],
                            start=(j == 0), stop=(j == jmax))
                # copy unnormalized out + l off PSUM so the banks free early
                for head in range(2):
                    nc.vector.tensor_copy(
                        avsb[2 * hp + head][:, ds(qt * 512, 512)],
                        pavs[head][:])

        def normalize(qh, avsb):
            # 1/l for all 8 heads at once via ACT: exp(-ln(l));
            # Ln and Exp share one table set -> no table switches
            q0 = qh * 1024
            lsb = lpool.tile([8, 1024], BF16, tag="lsb")
            for h in range(8):
                nc.sync.dma_start(lsb[ds(h, 1), :], avsb[h][ds(DH, 1), :])
            lnl = lpool.tile([8, 1024], F32, tag="lnl")
            nc.scalar.activation(lnl[:], lsb[:], LN)
            rinv = lpool.tile([8, 1024], BF16, tag="rinv")
            nc.scalar.activation(rinv[:], lnl[:], EXP, scale=-1.0)
            for h in range(8):
                hp, head = divmod(h, 2)
                rstage = lpool.tile([1, 1024], BF16, tag="rstage",
                                    name="rstage")
                nc.sync.dma_start(rstage[:], rinv[ds(h, 1), :])
                rb = lpool.tile([64, 1024], BF16, tag="rb")
                nc.gpsimd.partition_broadcast(rb[:], rstage[:], channels=64)
                if head == 0:
                    nc.gpsimd.tensor_mul(
                        ot[hp][ds(0, 64), ds(q0, 1024)],
                        avsb[h][ds(0, DH), :], rb[:])
                else:
                    ott = lpool.tile([64, 1024], BF16, tag="ott")
                    nc.gpsimd.tensor_mul(ott[:], avsb[h][ds(0, DH), :], rb[:])
                    nc.sync.dma_start(ot[hp][ds(64, 64), ds(q0, 1024)], ott[:])

        def out_proj(tt):
            for half in range(2):
                ps = ps512.tile([128, 512], F32, tag="p512")
                for k in range(4):
                    nc.tensor.matmul(
                        ps[:], ot[k][:, ts(tt, 128)],
                        wo_sb[k][:, ts(half, 512)],
                        start=(k == 0), stop=(k == 3))
                ysb = ypool.tile([128, 512], F32, tag="ysb")
                nc.vector.tensor_copy(ysb[:], ps[:])
                nc.sync.dma_start(y[ts(tt, 128), ds(half * 512, 512)], ysb[:])

        # ---------------- schedule ----------------------------------------
        avsb = [apool.tile([DH + 1, 1024], BF16, tag=f"av{h}", name=f"av{h}")
                for h in range(8)]
        for tt in range(8):           # v for token half 0
            v_proj(tt)
        for m in range(4):            # q/k half 0 + attention qh=0
            qk_proj(m, 0)
            qk_proj(4 + m, 0)
            attention(m, 0, avsb)
        normalize(0, avsb)
        for tt in range(8, 16):       # v for token half 1
            v_proj(tt)
        avsb = [apool.tile([DH + 1, 1024], BF16, tag=f"av{h}", name=f"avb{h}")
                for h in range(8)]
        for m in range(4):            # q/k half 1 + attention qh=1
            qk_proj(m, 1)
            qk_proj(4 + m, 1)
            attention(m, 1, avsb)
            out_proj(2 * m)           # fill PE while ACT works
            out_proj(2 * m + 1)
        normalize(1, avsb)
        for tt in range(8, 16):
            out_proj(tt)
    return nc


# ---------------- host side ------------------------------------------------

def _rope_tables():
    i = np.arange(DH // 2, dtype=np.float32)
    thetas = np.power(np.float32(10000.0), -2.0 * (i - 1.0) / DH)
    vals = thetas[:, None].astype(np.float32) * \
        np.arange(S, dtype=np.float32)[None, :]
    cos32 = np.cos(vals).astype(np.float32)
    sin32 = np.sin(vals).astype(np.float32)
    CC = np.tile(cos32, (4, 1))
    SSsw = np.concatenate([sin32, -sin32, sin32, -sin32], axis=0)
    return np.ascontiguousarray(CC), np.ascontiguousarray(SSsw)


def _qk_col_perm(g):
    cols = []
    for m in range(4):
        for hh in (2 * m, 2 * m + 1):
            hg = HL * g + hh
            cols += [hg * DH + 2 * i for i in range(32)]
            cols += [hg * DH + 2 * i + 1 for i in range(32)]
    return np.array(cols)


_CACHE = {}


def _get_module():
    if "nc" not in _CACHE:
        nc = bacc.Bacc("TRN2", target_bir_lowering=False, debug=False,
                       num_devices=8)
        build_kernel(nc)
        nc.compile()
        _CACHE["nc"] = nc
    return _CACHE["nc"]


def make_in_maps(x, Wqkv, Wout):
    bf = ml_dtypes.bfloat16
    x = np.asarray(x, np.float32)
    Wqkv = np.asarray(Wqkv, np.float32)
    Wout = np.asarray(Wout, np.float32)
    CC, SSsw = _rope_tables()
    shard = {}
    for g in range(2):
        perm = _qk_col_perm(g)
        vcols = np.arange(HL * g * DH, HL * (g + 1) * DH)
        shard[g] = dict(
            wq=np.ascontiguousarray(Wqkv[:, 0 * INNER:1 * INNER][:, perm].astype(bf)),
            wk=np.ascontiguousarray(Wqkv[:, 1 * INNER:2 * INNER][:, perm].astype(bf)),
            wv=np.ascontiguousarray(Wqkv[:, 2 * INNER:3 * INNER][:, vcols].astype(bf)),
            wo=np.ascontiguousarray(Wout[vcols, :].astype(bf)),
        )
    in_maps = []
    for c in range(8):
        b, g = c // 2, c % 2
        in_maps.append(dict(
            xT=np.ascontiguousarray(x[b].T.astype(bf)), cc=CC, ssw=SSsw,
            **shard[g]))
    return in_maps


def kernel(x, Wqkv, Wout, bout):
    bout = np.asarray(bout, np.float32)
    nc = _get_module()
    in_maps = make_in_maps(x, Wqkv, Wout)
    res = run_bass_kernel_spmd(nc, in_maps, core_ids=list(range(8)))
    ys = [r["y"] for r in res.results]
    out = np.stack([ys[2 * b] + ys[2 * b + 1] + bout for b in range(B)])
    return out.astype(np.float32)


# revision 22
# speedup vs baseline: 156.8221x; 1.8970x over previous
"""Trainium2 Bass kernel for nn_Attention (dense transformer block:
qkv proj + RoPE + causal attention + out proj), tensor-parallel over
8 NeuronCores: core c handles batch b=c//2, head-group g=c%2 (8 heads).

Self-contained: hardcodes all shapes; host preps transposed/permuted
bf16 shards, device computes partial y per core, host sums head-group
pairs and adds the output bias.

v2: bf16 matmul operands (fp32 psum accumulation), softmax 1/l via
ACT exp(-ln(l)) instead of DVE reciprocal, and attention/out-proj
interleaved with the projection stream so the PE never idles long
enough for the HAM clock gate to re-throttle.
"""

from contextlib import ExitStack

import numpy as np
import ml_dtypes

import concourse.bass as bass
import concourse.tile as tile
from concourse import bacc, mybir
from concourse.bass import ds, ts
from concourse.bass_utils import run_bass_kernel_spmd

B, S, D, H, DH = 4, 2048, 1024, 16, 64
HL = 8          # heads per core
INNER = H * DH  # 1024
KC = D // 128   # 8 contraction chunks
NT = S // 128   # 16 token tiles
F32 = mybir.dt.float32
BF16 = mybir.dt.bfloat16

EXP = mybir.ActivationFunctionType.Exp
LN = mybir.ActivationFunctionType.Ln
SCALE = 1.0 / np.sqrt(DH)


def build_kernel(nc):
    xT = nc.dram_tensor("xT", [D, S], BF16, kind="ExternalInput").ap()
    wq = nc.dram_tensor("wq", [D, HL * DH], BF16, kind="ExternalInput").ap()
    wk = nc.dram_tensor("wk", [D, HL * DH], BF16, kind="ExternalInput").ap()
    wv = nc.dram_tensor("wv", [D, HL * DH], BF16, kind="ExternalInput").ap()
    wo = nc.dram_tensor("wo", [HL * DH, D], BF16, kind="ExternalInput").ap()
    cc = nc.dram_tensor("cc", [128, S], F32, kind="ExternalInput").ap()
    ssw = nc.dram_tensor("ssw", [128, S], F32, kind="ExternalInput").ap()
    selm = nc.dram_tensor("selm", [8, HL * DH], BF16, kind="ExternalInput").ap()
    tmask = nc.dram_tensor("tmask", [128, 128], BF16, kind="ExternalInput").ap()
    y = nc.dram_tensor("y", [S, D], F32, kind="ExternalOutput").ap()

    with tile.TileContext(nc) as tc, ExitStack() as top:
        consts = top.enter_context(tc.tile_pool(name="consts", bufs=1))
        xtp = top.enter_context(tc.tile_pool(name="xtp", bufs=1))
        qkp = top.enter_context(tc.tile_pool(name="qkp", bufs=1))
        vpool = top.enter_context(tc.tile_pool(name="vpool", bufs=1))
        opool = top.enter_context(tc.tile_pool(name="opool", bufs=1))
        rtmp = top.enter_context(tc.tile_pool(name="rtmp", bufs=2))
        ppool = top.enter_context(tc.tile_pool(name="ppool", bufs=3))
        apool = top.enter_context(tc.tile_pool(name="apool", bufs=1))
        lpool = top.enter_context(tc.tile_pool(name="lpool", bufs=1))
        ypool = top.enter_context(tc.tile_pool(name="ypool", bufs=3))
        ps512 = top.enter_context(
            tc.tile_pool(name="ps512", bufs=2, space="PSUM"))
        pssc = top.enter_context(
            tc.tile_pool(name="pssc", bufs=2, space="PSUM"))
        psav = top.enter_context(
            tc.tile_pool(name="psav", bufs=1, space="PSUM"))

        # ---------------- constant / input loads --------------------------
        # order matters: wv + x-half0 first (v_proj starts the kernel),
        # wq/wk + rope tables next, wo (needed last) at the end
        wv_sb = consts.tile([128, KC, 512], BF16, tag="wv", name="wv")
        nc.sync.dma_start(wv_sb[:], wv.rearrange("(k p) n -> p k n", p=128))
        xth = [xtp.tile([128, S], BF16, tag=f"xth{k}", name=f"xth{k}")
               for k in range(KC)]
        for k in range(KC):
            nc.sync.dma_start(xth[k][:, 0:1024], xT[ts(k, 128), 0:1024])
        wq_sb = consts.tile([128, KC, 512], BF16, tag="wq", name="wq")
        nc.sync.dma_start(wq_sb[:], wq.rearrange("(k p) n -> p k n", p=128))
        wk_sb = consts.tile([128, KC, 512], BF16, tag="wk", name="wk")
        nc.sync.dma_start(wk_sb[:], wk.rearrange("(k p) n -> p k n", p=128))
        cc_sb = consts.tile([128, S], F32, tag="cc", name="cc")
        nc.sync.dma_start(cc_sb[:], cc[:])
        ssw_sb = consts.tile([128, S], F32, tag="ssw", name="ssw")
        nc.sync.dma_start(ssw_sb[:], ssw[:])
        for k in range(KC):
            nc.sync.dma_start(xth[k][:, 1024:2048], xT[ts(k, 128), 1024:2048])
        wo_sb = [consts.tile([128, D], BF16, tag=f"wo{k}", name=f"wo{k}")
                 for k in range(4)]
        for k in range(4):
            nc.sync.dma_start(wo_sb[k][:], wo[ts(k, 128), :])

        # split per S-half so proj half1 writes don't false-WAR against
        # attention qh0 reads of the same tile
        qkt = [[qkp.tile([128, 1024], BF16, tag=f"qkt{t}h{h}",
                         name=f"qkt{t}h{h}") for h in range(2)]
               for t in range(8)]
        vsb = [vpool.tile([128, 8, HL, DH + 1], BF16, tag=f"vsb{h}",
                          name=f"vsb{h}") for h in range(2)]
        for h in range(2):
            nc.gpsimd.memset(vsb[h][:, :, :, DH], 1.0)
        ot = [[opool.tile([128, 1024], BF16, tag=f"ot{m}q{qh}",
                          name=f"ot{m}q{qh}") for qh in range(2)]
              for m in range(4)]
        # constant strictly-upper-triangular zero mask for diagonal blocks
        trimask = consts.tile([128, 128], BF16, tag="trimask", name="trimask")
        nc.sync.dma_start(trimask[:], tmask[:])
        # head-select matrices: ssel[:, h, :] is e_h x ones(64) — used to
        # broadcast 1/l rows across 64 partitions via a tiny PE matmul
        ssel = consts.tile([8, 8, 64], BF16, tag="ssel", name="ssel")
        nc.sync.dma_start(ssel[:], selm.rearrange("p (h d) -> p h d", h=8))

        # ---------------- building blocks ---------------------------------
        def v_proj(tt):
            half, tsub = divmod(tt, 8)
            psV = ps512.tile([128, 512], F32, tag="p512")
            for k in range(KC):
                nc.tensor.matmul(
                    psV[:], xth[k][:, ds(tt * 128, 128)], wv_sb[:, k, :],
                    start=(k == 0), stop=(k == KC - 1))
            nc.vector.tensor_copy(
                vsb[half][:, tsub, :, 0:DH],
                psV[:].rearrange("p (h d) -> p h d", h=HL))

        def qk_proj(t, half):
            # q (t<4) / k (t>=4) projection + rope for 2 heads, one S-half
            wsrc = wq_sb if t < 4 else wk_sb
            m = t % 4
            tmp = rtmp.tile([128, 1024], BF16, tag="tmp")
            v2 = rtmp.tile([128, 1024], BF16, tag="v2")
            for piece in range(2):
                csl = ds(half * 1024 + piece * 512, 512)
                osl = ds(piece * 512, 512)
                ps = ps512.tile([128, 512], F32, tag="p512")
                for k in range(KC):
                    nc.tensor.matmul(
                        ps[:], wsrc[:, k, ts(m, 128)], xth[k][:, csl],
                        start=(k == 0), stop=(k == KC - 1))
                # rope: qkt = ps*CC + swap32(ps*SSsw)
                nc.vector.tensor_mul(tmp[:, osl], ps[:], cc_sb[:, csl])
                nc.vector.tensor_mul(v2[:, osl], ps[:], ssw_sb[:, csl])
            v2s = rtmp.tile([128, 1024], BF16, tag="v2s")
            for blk in range(4):
                nc.sync.dma_start(
                    v2s[ds(blk * 32, 32), :], v2[ds((blk ^ 1) * 32, 32), :])
            nc.vector.tensor_tensor(
                qkt[t][half][:], tmp[:], v2s[:], op=mybir.AluOpType.add)

        def attention(hp, qh, avsb):
            # heads (2hp, 2hp+1); query tokens [qh*1024, (qh+1)*1024)
            q0 = qh * 1024
            for qt in range(2):
                qt0 = q0 + qt * 512
                jmax = (qt0 + 512) // 128 - 1
                pavs = [psav.tile([DH + 1, 512], F32, tag=f"pav{head}",
                                  name=f"pav{head}")
                        for head in range(2)]
                for j in range(jmax + 1):
                    jh, jr = divmod(j, 8)
                    gs = max(qt0, 128 * j)
                    w = qt0 + 512 - gs
                    # both heads' scores share one 2-bank psum tile (head1 in
                    # bank 1 at offset 512) so one exp can cover both heads
                    sc = pssc.tile([128, 1024], F32, tag="sc")
                    for head in range(2):
                        nc.tensor.matmul(
                            sc[:, ds(head * 512, w)],
                            qkt[4 + hp][jh][ds(64 * head, 64),
                                            ds(128 * jr, 128)],
                            qkt[hp][qh][ds(64 * head, 64), ds(gs - q0, w)],
                            start=True, stop=True)
                    pj = ppool.tile([128, 1024], BF16, tag="pj")
                    if w == 512:
                        nc.scalar.activation(pj[:], sc[:], EXP, scale=SCALE)
                    else:
                        for head in range(2):
                            nc.scalar.activation(
                                pj[:, ds(head * 512, w)],
                                sc[:, ds(head * 512, w)], EXP, scale=SCALE)
                    if gs == 128 * j:
                        # diagonal block: causal-mask first 128 cols per head
                        for head in range(2):
                            nc.vector.tensor_mul(
                                pj[:, ds(head * 512, 128)],
                                pj[:, ds(head * 512, 128)], trimask[:])
                    for head in range(2):
                        nc.tensor.matmul(
                            pavs[head][:, ds(gs - qt0, w)],
                            vsb[jh][:, jr, 2 * hp + head, :],
                            pj[:, ds(head * 512, w)],
                            start=(j == 0), stop=(j == jmax))
                # copy unnormalized out + l off PSUM so the banks free early
                for head in range(2):
                    nc.vector.tensor_copy(
                        avsb[2 * hp + head][:, ds(qt * 512, 512)],
                        pavs[head][:])

        def normalize(qh, avsb):
            # 1/l for all 8 heads at once via ACT: exp(-ln(l));
            # Ln and Exp share one table set -> no table switches
            lsb = lpool.tile([8, 1024], BF16, tag="lsb")
            for h in range(8):
                nc.sync.dma_start(lsb[ds(h, 1), :], avsb[h][ds(DH, 1), :])
            lnl = lpool.tile([8, 1024], F32, tag="lnl")
            nc.scalar.activation(lnl[:], lsb[:], LN)
            rinv = lpool.tile([8, 1024], BF16, tag="rinv")
            nc.scalar.activation(rinv[:], lnl[:], EXP, scale=-1.0)
            for hp in range(4):
                # broadcast 1/l across 64 partitions with a tiny PE matmul:
                # rb[64, 1024] = ssel[:, h, :]^T @ rinv
                rbs = []
                for head in range(2):
                    rb = pssc.tile([64, 1024], F32, tag="sc",
                                   name=f"rb{qh}{hp}{head}")
                    for piece in range(2):
                        nc.tensor.matmul(
                            rb[:, ds(piece * 512, 512)],
                            ssel[:, 2 * hp + head, :],
                            rinv[:, ds(piece * 512, 512)],
                            start=True, stop=True)
                    rbs.append(rb)
                nc.vector.tensor_mul(
                    ot[hp][qh][ds(0, 64), :],
                    avsb[2 * hp][ds(0, DH), :], rbs[0][:])
                ott = lpool.tile([64, 1024], BF16, tag="ott")
                nc.vector.tensor_mul(
                    ott[:], avsb[2 * hp + 1][ds(0, DH), :], rbs[1][:])
                nc.sync.dma_start(ot[hp][qh][ds(64, 64), :], ott[:])

        def out_proj(tt):
            qh, tsub = divmod(tt, 8)
            for half in range(2):
                ps = ps512.tile([128, 512], F32, tag="p512")
                for k in range(4):
                    nc.tensor.matmul(
                        ps[:], ot[k][qh][:, ts(tsub, 128)],
                        wo_sb[k][:, ts(half, 512)],
                        start=(k == 0), stop=(k == 3))
                ysb = ypool.tile([128, 512], F32, tag="ysb")
                nc.vector.tensor_copy(ysb[:], ps[:])
                nc.sync.dma_start(y[ts(tt, 128), ds(half * 512, 512)], ysb[:])

        # ---------------- schedule ----------------------------------------
        avsb = [apool.tile([DH + 1, 1024], BF16, tag=f"av{h}", name=f"av{h}")
                for h in range(8)]
        for tt in range(8):           # v for token half 0
            v_proj(tt)
        for m in range(4):            # q/k half 0 + attention qh=0
            qk_proj(m, 0)
            qk_proj(4 + m, 0)
            attention(m, 0, avsb)
        normalize(0, avsb)
        for tt in range(8, 16):       # v for token half 1
            v_proj(tt)
        avsb = [apool.tile([DH + 1, 1024], BF16, tag=f"av{h}", name=f"avb{h}")
                for h in range(8)]
        for m in range(4):            # q/k half 1 + attention qh=1
            qk_proj(m, 1)
            qk_proj(4 + m, 1)
            attention(m, 1, avsb)
            out_proj(2 * m)           # fill PE while ACT works
            out_proj(2 * m + 1)
        normalize(1, avsb)
        for tt in range(8, 16):
            out_proj(tt)
    return nc


# ---------------- host side ------------------------------------------------

def _rope_tables():
    i = np.arange(DH // 2, dtype=np.float32)
    thetas = np.power(np.float32(10000.0), -2.0 * (i - 1.0) / DH)
    vals = thetas[:, None].astype(np.float32) * \
        np.arange(S, dtype=np.float32)[None, :]
    cos32 = np.cos(vals).astype(np.float32)
    sin32 = np.sin(vals).astype(np.float32)
    CC = np.tile(cos32, (4, 1))
    SSsw = np.concatenate([sin32, -sin32, sin32, -sin32], axis=0)
    return np.ascontiguousarray(CC), np.ascontiguousarray(SSsw)


def _qk_col_perm(g):
    cols = []
    for m in range(4):
        for hh in (2 * m, 2 * m + 1):
            hg = HL * g + hh
            cols += [hg * DH + 2 * i for i in range(32)]
            cols += [hg * DH + 2 * i + 1 for i in range(32)]
    return np.array(cols)


_CACHE = {}


def _get_module():
    if "nc" not in _CACHE:
        nc = bacc.Bacc("TRN2", target_bir_lowering=False, debug=False,
                       num_devices=8)
        build_kernel(nc)
        nc.compile()
        _CACHE["nc"] = nc
    return _CACHE["nc"]


def make_in_maps(x, Wqkv, Wout):
    bf = ml_dtypes.bfloat16
    x = np.asarray(x, np.float32)
    Wqkv = np.asarray(Wqkv, np.float32)
    Wout = np.asarray(Wout, np.float32)
    CC, SSsw = _rope_tables()
    shard = {}
    for g in range(2):
        perm = _qk_col_perm(g)
        vcols = np.arange(HL * g * DH, HL * (g + 1) * DH)
        shard[g] = dict(
            wq=np.ascontiguousarray(Wqkv[:, 0 * INNER:1 * INNER][:, perm].astype(bf)),
            wk=np.ascontiguousarray(Wqkv[:, 1 * INNER:2 * INNER][:, perm].astype(bf)),
            wv=np.ascontiguousarray(Wqkv[:, 2 * INNER:3 * INNER][:, vcols].astype(bf)),
            wo=np.ascontiguousarray(Wout[vcols, :].astype(bf)),
        )
    selm = np.zeros((8, 8, 64), np.float32)
    for h in range(8):
        selm[h, h, :] = 1.0
    selm = np.ascontiguousarray(selm.reshape(8, 512).astype(bf))
    tmask = np.ascontiguousarray(
        np.triu(np.ones((128, 128), np.float32)).astype(bf))
    in_maps = []
    for c in range(8):
        b, g = c // 2, c % 2
        in_maps.append(dict(
            xT=np.ascontiguousarray(x[b].T.astype(bf)), cc=CC, ssw=SSsw,
            selm=selm, tmask=tmask, **shard[g]))
    return in_maps


def kernel(x, Wqkv, Wout, bout):
    bout = np.asarray(bout, np.float32)
    nc = _get_module()
    in_maps = make_in_maps(x, Wqkv, Wout)
    res = run_bass_kernel_spmd(nc, in_maps, core_ids=list(range(8)))
    ys = [r["y"] for r in res.results]
    out = np.stack([ys[2 * b] + ys[2 * b + 1] + bout for b in range(B)])
    return out.astype(np.float32)


# revision 28
# speedup vs baseline: 182.3510x; 1.1628x over previous
"""Trainium2 Bass kernel for nn_Attention (dense transformer block:
qkv proj + RoPE + causal attention + out proj), tensor-parallel over
8 NeuronCores: core c handles batch b=c//2, head-group g=c%2 (8 heads).

Self-contained: hardcodes all shapes; host preps transposed/permuted
bf16 shards, device computes partial y per core, host sums head-group
pairs and adds the output bias.

v2: bf16 matmul operands (fp32 psum accumulation), softmax 1/l via
ACT exp(-ln(l)) instead of DVE reciprocal, and attention/out-proj
interleaved with the projection stream so the PE never idles long
enough for the HAM clock gate to re-throttle.
"""

from contextlib import ExitStack

import numpy as np
import ml_dtypes

import concourse.bass as bass
import concourse.tile as tile
from concourse import bacc, mybir
from concourse.bass import ds, ts
from concourse.bass_utils import run_bass_kernel_spmd

B, S, D, H, DH = 4, 2048, 1024, 16, 64
HL = 8          # heads per core
INNER = H * DH  # 1024
KC = D // 128   # 8 contraction chunks
NT = S // 128   # 16 token tiles
F32 = mybir.dt.float32
BF16 = mybir.dt.bfloat16

EXP = mybir.ActivationFunctionType.Exp
LN = mybir.ActivationFunctionType.Ln
SCALE = 1.0 / np.sqrt(DH)


def build_kernel(nc):
    xT = nc.dram_tensor("xT", [D, S], BF16, kind="ExternalInput").ap()
    wq = nc.dram_tensor("wq", [D, HL * DH], BF16, kind="ExternalInput").ap()
    wk = nc.dram_tensor("wk", [D, HL * DH], BF16, kind="ExternalInput").ap()
    wv = nc.dram_tensor("wv", [D, HL * DH], BF16, kind="ExternalInput").ap()
    wo = nc.dram_tensor("wo", [HL * DH, D], BF16, kind="ExternalInput").ap()
    cc = nc.dram_tensor("cc", [128, S], F32, kind="ExternalInput").ap()
    ssw = nc.dram_tensor("ssw", [128, S], F32, kind="ExternalInput").ap()
    selm = nc.dram_tensor("selm", [8, HL * DH], BF16, kind="ExternalInput").ap()
    tmask = nc.dram_tensor("tmask", [128, 128], BF16, kind="ExternalInput").ap()
    y = nc.dram_tensor("y", [S, D], F32, kind="ExternalOutput").ap()

    with tile.TileContext(nc) as tc, ExitStack() as top:
        consts = top.enter_context(tc.tile_pool(name="consts", bufs=1))
        xtp = top.enter_context(tc.tile_pool(name="xtp", bufs=1))
        qkp = top.enter_context(tc.tile_pool(name="qkp", bufs=1))
        vpool = top.enter_context(tc.tile_pool(name="vpool", bufs=1))
        opool = top.enter_context(tc.tile_pool(name="opool", bufs=1))
        rtmp = top.enter_context(tc.tile_pool(name="rtmp", bufs=2))
        ppool = top.enter_context(tc.tile_pool(name="ppool", bufs=3))
        apool = top.enter_context(tc.tile_pool(name="apool", bufs=1))
        lpool = top.enter_context(tc.tile_pool(name="lpool", bufs=1))
        ypool = top.enter_context(tc.tile_pool(name="ypool", bufs=3))
        ps512 = top.enter_context(
            tc.tile_pool(name="ps512", bufs=2, space="PSUM"))
        pssc = top.enter_context(
            tc.tile_pool(name="pssc", bufs=2, space="PSUM"))
        psav = top.enter_context(
            tc.tile_pool(name="psav", bufs=1, space="PSUM"))

        # ---------------- constant / input loads --------------------------
        # order matters: wv + x-half0 first (v_proj starts the kernel),
        # wq/wk + rope tables next, wo (needed last) at the end
        wv_sb = consts.tile([128, KC, 512], BF16, tag="wv", name="wv")
        nc.sync.dma_start(wv_sb[:], wv.rearrange("(k p) n -> p k n", p=128))
        xth = [xtp.tile([128, S], BF16, tag=f"xth{k}", name=f"xth{k}")
               for k in range(KC)]
        for k in range(KC):
            nc.sync.dma_start(xth[k][:, 0:1024], xT[ts(k, 128), 0:1024])
        wq_sb = consts.tile([128, KC, 512], BF16, tag="wq", name="wq")
        nc.sync.dma_start(wq_sb[:], wq.rearrange("(k p) n -> p k n", p=128))
        wk_sb = consts.tile([128, KC, 512], BF16, tag="wk", name="wk")
        nc.sync.dma_start(wk_sb[:], wk.rearrange("(k p) n -> p k n", p=128))
        cc_sb = consts.tile([128, S], F32, tag="cc", name="cc")
        nc.sync.dma_start(cc_sb[:], cc[:])
        ssw_sb = consts.tile([128, S], F32, tag="ssw", name="ssw")
        nc.sync.dma_start(ssw_sb[:], ssw[:])
        for k in range(KC):
            nc.sync.dma_start(xth[k][:, 1024:2048], xT[ts(k, 128), 1024:2048])
        wo_sb = [consts.tile([128, D], BF16, tag=f"wo{k}", name=f"wo{k}")
                 for k in range(4)]
        for k in range(4):
            nc.sync.dma_start(wo_sb[k][:], wo[ts(k, 128), :])

        # split per S-half so proj half1 writes don't false-WAR against
        # attention qh0 reads of the same tile
        qkt = [[qkp.tile([128, 1024], BF16, tag=f"qkt{t}h{h}",
                         name=f"qkt{t}h{h}") for h in range(2)]
               for t in range(8)]
        vsb = [vpool.tile([128, 8, HL, DH + 1], BF16, tag=f"vsb{h}",
                          name=f"vsb{h}") for h in range(2)]
        for h in range(2):
            nc.gpsimd.memset(vsb[h][:, :, :, DH], 1.0)
        ot = [[opool.tile([128, 1024], BF16, tag=f"ot{m}q{qh}",
                          name=f"ot{m}q{qh}") for qh in range(2)]
              for m in range(4)]
        # constant strictly-upper-triangular zero mask for diagonal blocks
        trimask = consts.tile([128, 128], BF16, tag="trimask", name="trimask")
        nc.sync.dma_start(trimask[:], tmask[:])
        # head-select matrices: ssel[:, h, :] is e_h x ones(64) — used to
        # broadcast 1/l rows across 64 partitions via a tiny PE matmul
        ssel = consts.tile([8, 8, 64], BF16, tag="ssel", name="ssel")
        nc.sync.dma_start(ssel[:], selm.rearrange("p (h d) -> p h d", h=8))

        # PE warm-up during the DMA prologue: ~3 fp32 matmuls (~5us of PE
        # activity) trip the HAM clock gate to K=8/8 before the real work
        # arrives; the result lands in y[0:128, 0:512] and is overwritten
        # by out_proj(0) later on the same DMA queue (ordered).
        warm = pssc.tile([128, 512], F32, tag="sc", name="warm")
        for i in range(3):
            nc.tensor.matmul(warm[:], cc_sb[:, 0:128], cc_sb[:, 0:512],
                             start=(i == 0), stop=(i == 2))
        wsb = ypool.tile([128, 512], F32, tag="ysb", name="wsb")
        nc.vector.tensor_copy(wsb[:], warm[:])
        nc.sync.dma_start(y[0:128, 0:512], wsb[:])

        # ---------------- building blocks ---------------------------------
        def v_proj(tt):
            half, tsub = divmod(tt, 8)
            psV = ps512.tile([128, 512], F32, tag="p512")
            for k in range(KC):
                nc.tensor.matmul(
                    psV[:], xth[k][:, ds(tt * 128, 128)], wv_sb[:, k, :],
                    start=(k == 0), stop=(k == KC - 1))
            nc.vector.tensor_copy(
                vsb[half][:, tsub, :, 0:DH],
                psV[:].rearrange("p (h d) -> p h d", h=HL))

        def qk_proj(t, half):
            # q (t<4) / k (t>=4) projection + rope for 2 heads, one S-half
            wsrc = wq_sb if t < 4 else wk_sb
            m = t % 4
            tmp = rtmp.tile([128, 1024], BF16, tag="tmp")
            v2 = rtmp.tile([128, 1024], BF16, tag="v2")
            for piece in range(2):
                csl = ds(half * 1024 + piece * 512, 512)
                osl = ds(piece * 512, 512)
                ps = ps512.tile([128, 512], F32, tag="p512")
                for k in range(KC):
                    nc.tensor.matmul(
                        ps[:], wsrc[:, k, ts(m, 128)], xth[k][:, csl],
                        start=(k == 0), stop=(k == KC - 1))
                # rope: qkt = ps*CC + swap32(ps*SSsw)
                nc.vector.tensor_mul(tmp[:, osl], ps[:], cc_sb[:, csl])
                nc.vector.tensor_mul(v2[:, osl], ps[:], ssw_sb[:, csl])
            v2s = rtmp.tile([128, 1024], BF16, tag="v2s")
            for blk in range(4):
                nc.sync.dma_start(
                    v2s[ds(blk * 32, 32), :], v2[ds((blk ^ 1) * 32, 32), :])
            nc.vector.tensor_tensor(
                qkt[t][half][:], tmp[:], v2s[:], op=mybir.AluOpType.add)

        def attention(hp, qh, avsb, lsb):
            # heads (2hp, 2hp+1); query tokens [qh*1024, (qh+1)*1024)
            q0 = qh * 1024
            for qt in range(2):
                qt0 = q0 + qt * 512
                jmax = (qt0 + 512) // 128 - 1
                pavs = [psav.tile([DH + 1, 512], F32, tag=f"pav{head}",
                                  name=f"pav{head}")
                        for head in range(2)]
                for j in range(jmax + 1):
                    jh, jr = divmod(j, 8)
                    gs = max(qt0, 128 * j)
                    w = qt0 + 512 - gs
                    # both heads' scores share one 2-bank psum tile (head1 in
                    # bank 1 at offset 512) so one exp can cover both heads
                    sc = pssc.tile([128, 1024], F32, tag="sc")
                    for head in range(2):
                        nc.tensor.matmul(
                            sc[:, ds(head * 512, w)],
                            qkt[4 + hp][jh][ds(64 * head, 64),
                                            ds(128 * jr, 128)],
                            qkt[hp][qh][ds(64 * head, 64), ds(gs - q0, w)],
                            start=True, stop=True)
                    pj = ppool.tile([128, 1024], BF16, tag="pj")
                    if w == 512:
                        nc.scalar.activation(pj[:], sc[:], EXP, scale=SCALE)
                    else:
                        for head in range(2):
                            nc.scalar.activation(
                                pj[:, ds(head * 512, w)],
                                sc[:, ds(head * 512, w)], EXP, scale=SCALE)
                    if gs == 128 * j:
                        # diagonal block: causal-mask first 128 cols per head
                        for head in range(2):
                            nc.vector.tensor_mul(
                                pj[:, ds(head * 512, 128)],
                                pj[:, ds(head * 512, 128)], trimask[:])
                    for head in range(2):
                        nc.tensor.matmul(
                            pavs[head][:, ds(gs - qt0, w)],
                            vsb[jh][:, jr, 2 * hp + head, :],
                            pj[:, ds(head * 512, w)],
                            start=(j == 0), stop=(j == jmax))
                # copy unnormalized out + l off PSUM so the banks free early
                for head in range(2):
                    nc.vector.tensor_copy(
                        avsb[2 * hp + head][:, ds(qt * 512, 512)],
                        pavs[head][:])
            # gather this pair's softmax denominators early so the final
            # normalize has no DMA chain left to wait on
            for head in range(2):
                h = 2 * hp + head
                nc.sync.dma_start(lsb[ds(h, 1), :], avsb[h][ds(DH, 1), :])

        def normalize(qh, avsb, lsb):
            # 1/l for all 8 heads at once via ACT: exp(-ln(l));
            # Ln and Exp share one table set -> no table switches
            lnl = lpool.tile([8, 1024], F32, tag="lnl")
            nc.scalar.activation(lnl[:], lsb[:], LN)
            rinv = lpool.tile([8, 1024], BF16, tag="rinv")
            nc.scalar.activation(rinv[:], lnl[:], EXP, scale=-1.0)
            for hp in range(4):
                # broadcast 1/l across 64 partitions with a tiny PE matmul:
                # rb[64, 1024] = ssel[:, h, :]^T @ rinv
                rbs = []
                for head in range(2):
                    rb = pssc.tile([64, 1024], F32, tag="sc",
                                   name=f"rb{qh}{hp}{head}")
                    for piece in range(2):
                        nc.tensor.matmul(
                            rb[:, ds(piece * 512, 512)],
                            ssel[:, 2 * hp + head, :],
                            rinv[:, ds(piece * 512, 512)],
                            start=True, stop=True)
                    rbs.append(rb)
                nc.vector.tensor_mul(
                    ot[hp][qh][ds(0, 64), :],
                    avsb[2 * hp][ds(0, DH), :], rbs[0][:])
                ott = lpool.tile([64, 1024], BF16, tag="ott")
                nc.vector.tensor_mul(
                    ott[:], avsb[2 * hp + 1][ds(0, DH), :], rbs[1][:])
                nc.sync.dma_start(ot[hp][qh][ds(64, 64), :], ott[:])

        def out_proj(tt):
            qh, tsub = divmod(tt, 8)
            for half in range(2):
                ps = ps512.tile([128, 512], F32, tag="p512")
                for k in range(4):
                    nc.tensor.matmul(
                        ps[:], ot[k][qh][:, ts(tsub, 128)],
                        wo_sb[k][:, ts(half, 512)],
                        start=(k == 0), stop=(k == 3))
                ysb = ypool.tile([128, 512], F32, tag="ysb")
                nc.vector.tensor_copy(ysb[:], ps[:])
                nc.sync.dma_start(y[ts(tt, 128), ds(half * 512, 512)], ysb[:])

        # ---------------- schedule ----------------------------------------
        avsb = [apool.tile([DH + 1, 1024], BF16, tag=f"av{h}", name=f"av{h}")
                for h in range(8)]
        lsb = lpool.tile([8, 1024], BF16, tag="lsb", name="lsb0")
        for tt in range(8):           # v for token half 0
            v_proj(tt)
        for m in range(4):            # q/k half 0 + attention qh=0
            qk_proj(m, 0)
            qk_proj(4 + m, 0)
            attention(m, 0, avsb, lsb)
        normalize(0, avsb, lsb)
        for tt in range(8, 16):       # v for token half 1
            v_proj(tt)
        avsb = [apool.tile([DH + 1, 1024], BF16, tag=f"av{h}", name=f"avb{h}")
                for h in range(8)]
        lsb = lpool.tile([8, 1024], BF16, tag="lsb", name="lsb1")
        for m in range(4):            # q/k half 1 + attention qh=1
            qk_proj(m, 1)
            qk_proj(4 + m, 1)
            attention(m, 1, avsb, lsb)
            out_proj(2 * m)           # fill PE while ACT works
            out_proj(2 * m + 1)
        normalize(1, avsb, lsb)
        for tt in range(8, 16):
            out_proj(tt)
    return nc


# ---------------- host side ------------------------------------------------

def _rope_tables():
    i = np.arange(DH // 2, dtype=np.float32)
    thetas = np.power(np.float32(10000.0), -2.0 * (i - 1.0) / DH)
    vals = thetas[:, None].astype(np.float32) * \
        np.arange(S, dtype=np.float32)[None, :]
    cos32 = np.cos(vals).astype(np.float32)
    sin32 = np.sin(vals).astype(np.float32)
    CC = np.tile(cos32, (4, 1))
    SSsw = np.concatenate([sin32, -sin32, sin32, -sin32], axis=0)
    return np.ascontiguousarray(CC), np.ascontiguousarray(SSsw)


def _qk_col_perm(g):
    cols = []
    for m in range(4):
        for hh in (2 * m, 2 * m + 1):
            hg = HL * g + hh
            cols += [hg * DH + 2 * i for i in range(32)]
            cols += [hg * DH + 2 * i + 1 for i in range(32)]
    return np.array(cols)


_CACHE = {}


def _get_module():
    if "nc" not in _CACHE:
        nc = bacc.Bacc("TRN2", target_bir_lowering=False, debug=False,
                       num_devices=8)
        build_kernel(nc)
        nc.compile()
        _CACHE["nc"] = nc
    return _CACHE["nc"]


def make_in_maps(x, Wqkv, Wout):
    bf = ml_dtypes.bfloat16
    x = np.asarray(x, np.float32)
    Wqkv = np.asarray(Wqkv, np.float32)
    Wout = np.asarray(Wout, np.float32)
    CC, SSsw = _rope_tables()
    shard = {}
    for g in range(2):
        perm = _qk_col_perm(g)
        vcols = np.arange(HL * g * DH, HL * (g + 1) * DH)
        shard[g] = dict(
            wq=np.ascontiguousarray(Wqkv[:, 0 * INNER:1 * INNER][:, perm].astype(bf)),
            wk=np.ascontiguousarray(Wqkv[:, 1 * INNER:2 * INNER][:, perm].astype(bf)),
            wv=np.ascontiguousarray(Wqkv[:, 2 * INNER:3 * INNER][:, vcols].astype(bf)),
            wo=np.ascontiguousarray(Wout[vcols, :].astype(bf)),
        )
    selm = np.zeros((8, 8, 64), np.float32)
    for h in range(8):
        selm[h, h, :] = 1.0
    selm = np.ascontiguousarray(selm.reshape(8, 512).astype(bf))
    tmask = np.ascontiguousarray(
        np.triu(np.ones((128, 128), np.float32)).astype(bf))
    in_maps = []
    for c in range(8):
        b, g = c // 2, c % 2
        in_maps.append(dict(
            xT=np.ascontiguousarray(x[b].T.astype(bf)), cc=CC, ssw=SSsw,
            selm=selm, tmask=tmask, **shard[g]))
    return in_maps


def kernel(x, Wqkv, Wout, bout):
    bout = np.asarray(bout, np.float32)
    nc = _get_module()
    in_maps = make_in_maps(x, Wqkv, Wout)
    res = run_bass_kernel_spmd(nc, in_maps, core_ids=list(range(8)))
    ys = [r["y"] for r in res.results]
    out = np.stack([ys[2 * b] + ys[2 * b + 1] + bout for b in range(B)])
    return out.astype(np.float32)


# revision 29
# speedup vs baseline: 182.6105x; 1.0014x over previous
"""Trainium2 Bass kernel for nn_Attention (dense transformer block:
qkv proj + RoPE + causal attention + out proj), tensor-parallel over
8 NeuronCores: core c handles batch b=c//2, head-group g=c%2 (8 heads).

Self-contained: hardcodes all shapes; host preps transposed/permuted
bf16 shards, device computes partial y per core, host sums head-group
pairs and adds the output bias.

v2: bf16 matmul operands (fp32 psum accumulation), softmax 1/l via
ACT exp(-ln(l)) instead of DVE reciprocal, and attention/out-proj
interleaved with the projection stream so the PE never idles long
enough for the HAM clock gate to re-throttle.
"""

from contextlib import ExitStack

import numpy as np
import ml_dtypes

import concourse.bass as bass
import concourse.tile as tile
from concourse import bacc, mybir
from concourse.bass import ds, ts
from concourse.bass_utils import run_bass_kernel_spmd

B, S, D, H, DH = 4, 2048, 1024, 16, 64
HL = 8          # heads per core
INNER = H * DH  # 1024
KC = D // 128   # 8 contraction chunks
NT = S // 128   # 16 token tiles
F32 = mybir.dt.float32
BF16 = mybir.dt.bfloat16

EXP = mybir.ActivationFunctionType.Exp
LN = mybir.ActivationFunctionType.Ln
SCALE = 1.0 / np.sqrt(DH)


def build_kernel(nc):
    xT = nc.dram_tensor("xT", [D, S], BF16, kind="ExternalInput").ap()
    wq = nc.dram_tensor("wq", [D, HL * DH], BF16, kind="ExternalInput").ap()
    wk = nc.dram_tensor("wk", [D, HL * DH], BF16, kind="ExternalInput").ap()
    wv = nc.dram_tensor("wv", [D, HL * DH], BF16, kind="ExternalInput").ap()
    wo = nc.dram_tensor("wo", [HL * DH, D], BF16, kind="ExternalInput").ap()
    cc = nc.dram_tensor("cc", [128, S], F32, kind="ExternalInput").ap()
    ssw = nc.dram_tensor("ssw", [128, S], F32, kind="ExternalInput").ap()
    selm = nc.dram_tensor("selm", [8, HL * DH], BF16, kind="ExternalInput").ap()
    tmask = nc.dram_tensor("tmask", [128, 128], BF16, kind="ExternalInput").ap()
    y = nc.dram_tensor("y", [S, D], F32, kind="ExternalOutput").ap()

    with tile.TileContext(nc) as tc, ExitStack() as top:
        consts = top.enter_context(tc.tile_pool(name="consts", bufs=1))
        xtp = top.enter_context(tc.tile_pool(name="xtp", bufs=1))
        qkp = top.enter_context(tc.tile_pool(name="qkp", bufs=1))
        vpool = top.enter_context(tc.tile_pool(name="vpool", bufs=1))
        opool = top.enter_context(tc.tile_pool(name="opool", bufs=1))
        rtmp = top.enter_context(tc.tile_pool(name="rtmp", bufs=2))
        ppool = top.enter_context(tc.tile_pool(name="ppool", bufs=4))
        apool = top.enter_context(tc.tile_pool(name="apool", bufs=1))
        lpool = top.enter_context(tc.tile_pool(name="lpool", bufs=2))
        ypool = top.enter_context(tc.tile_pool(name="ypool", bufs=3))
        ps512 = top.enter_context(
            tc.tile_pool(name="ps512", bufs=2, space="PSUM"))
        pssc = top.enter_context(
            tc.tile_pool(name="pssc", bufs=2, space="PSUM"))
        psav = top.enter_context(
            tc.tile_pool(name="psav", bufs=1, space="PSUM"))

        # ---------------- constant / input loads --------------------------
        # order matters: wv + x-half0 first (v_proj starts the kernel),
        # wq/wk + rope tables next, wo (needed last) at the end
        wv_sb = consts.tile([128, KC, 512], BF16, tag="wv", name="wv")
        nc.sync.dma_start(wv_sb[:], wv.rearrange("(k p) n -> p k n", p=128))
        xth = [xtp.tile([128, S], BF16, tag=f"xth{k}", name=f"xth{k}")
               for k in range(KC)]
        for k in range(KC):
            nc.sync.dma_start(xth[k][:, 0:1024], xT[ts(k, 128), 0:1024])
        wq_sb = consts.tile([128, KC, 512], BF16, tag="wq", name="wq")
        nc.sync.dma_start(wq_sb[:], wq.rearrange("(k p) n -> p k n", p=128))
        wk_sb = consts.tile([128, KC, 512], BF16, tag="wk", name="wk")
        nc.sync.dma_start(wk_sb[:], wk.rearrange("(k p) n -> p k n", p=128))
        cc_sb = consts.tile([128, S], F32, tag="cc", name="cc")
        nc.sync.dma_start(cc_sb[:], cc[:])
        ssw_sb = consts.tile([128, S], F32, tag="ssw", name="ssw")
        nc.sync.dma_start(ssw_sb[:], ssw[:])
        for k in range(KC):
            nc.sync.dma_start(xth[k][:, 1024:2048], xT[ts(k, 128), 1024:2048])
        wo_sb = [consts.tile([128, D], BF16, tag=f"wo{k}", name=f"wo{k}")
                 for k in range(4)]
        for k in range(4):
            nc.sync.dma_start(wo_sb[k][:], wo[ts(k, 128), :])

        # split per S-half so proj half1 writes don't false-WAR against
        # attention qh0 reads of the same tile
        qkt = [[qkp.tile([128, 1024], BF16, tag=f"qkt{t}h{h}",
                         name=f"qkt{t}h{h}") for h in range(2)]
               for t in range(8)]
        vsb = [vpool.tile([128, 8, HL, DH + 1], BF16, tag=f"vsb{h}",
                          name=f"vsb{h}") for h in range(2)]
        for h in range(2):
            nc.gpsimd.memset(vsb[h][:, :, :, DH], 1.0)
        ot = [[opool.tile([128, 1024], BF16, tag=f"ot{m}q{qh}",
                          name=f"ot{m}q{qh}") for qh in range(2)]
              for m in range(4)]
        # constant strictly-upper-triangular zero mask for diagonal blocks
        trimask = consts.tile([128, 128], BF16, tag="trimask", name="trimask")
        nc.sync.dma_start(trimask[:], tmask[:])
        # head-select matrices: ssel[:, h, :] is e_h x ones(64) — used to
        # broadcast 1/l rows across 64 partitions via a tiny PE matmul
        ssel = consts.tile([8, 8, 64], BF16, tag="ssel", name="ssel")
        nc.sync.dma_start(ssel[:], selm.rearrange("p (h d) -> p h d", h=8))

        # PE warm-up during the DMA prologue: ~3 fp32 matmuls (~5us of PE
        # activity) trip the HAM clock gate to K=8/8 before the real work
        # arrives; the result lands in y[0:128, 0:512] and is overwritten
        # by out_proj(0) later on the same DMA queue (ordered).
        warm = pssc.tile([128, 512], F32, tag="sc", name="warm")
        for i in range(3):
            nc.tensor.matmul(warm[:], cc_sb[:, 0:128], cc_sb[:, 0:512],
                             start=(i == 0), stop=(i == 2))
        wsb = ypool.tile([128, 512], F32, tag="ysb", name="wsb")
        nc.vector.tensor_copy(wsb[:], warm[:])
        nc.sync.dma_start(y[0:128, 0:512], wsb[:])

        # ---------------- building blocks ---------------------------------
        def v_proj(tt):
            half, tsub = divmod(tt, 8)
            psV = ps512.tile([128, 512], F32, tag="p512")
            for k in range(KC):
                nc.tensor.matmul(
                    psV[:], xth[k][:, ds(tt * 128, 128)], wv_sb[:, k, :],
                    start=(k == 0), stop=(k == KC - 1))
            nc.vector.tensor_copy(
                vsb[half][:, tsub, :, 0:DH],
                psV[:].rearrange("p (h d) -> p h d", h=HL))

        def qk_proj(t, half):
            # q (t<4) / k (t>=4) projection + rope for 2 heads, one S-half
            wsrc = wq_sb if t < 4 else wk_sb
            m = t % 4
            tmp = rtmp.tile([128, 1024], BF16, tag="tmp")
            v2 = rtmp.tile([128, 1024], BF16, tag="v2")
            for piece in range(2):
                csl = ds(half * 1024 + piece * 512, 512)
                osl = ds(piece * 512, 512)
                ps = ps512.tile([128, 512], F32, tag="p512")
                for k in range(KC):
                    nc.tensor.matmul(
                        ps[:], wsrc[:, k, ts(m, 128)], xth[k][:, csl],
                        start=(k == 0), stop=(k == KC - 1))
                # rope: qkt = ps*CC + swap32(ps*SSsw)
                nc.vector.tensor_mul(tmp[:, osl], ps[:], cc_sb[:, csl])
                nc.vector.tensor_mul(v2[:, osl], ps[:], ssw_sb[:, csl])
            v2s = rtmp.tile([128, 1024], BF16, tag="v2s")
            for blk in range(4):
                nc.sync.dma_start(
                    v2s[ds(blk * 32, 32), :], v2[ds((blk ^ 1) * 32, 32), :])
            nc.vector.tensor_tensor(
                qkt[t][half][:], tmp[:], v2s[:], op=mybir.AluOpType.add)

        def attention(hp, qh, avsb, lsb):
            # heads (2hp, 2hp+1); query tokens [qh*1024, (qh+1)*1024)
            q0 = qh * 1024
            for qt in range(2):
                qt0 = q0 + qt * 512
                jmax = (qt0 + 512) // 128 - 1
                pavs = [psav.tile([DH + 1, 512], F32, tag=f"pav{head}",
                                  name=f"pav{head}")
                        for head in range(2)]
                for j in range(jmax + 1):
                    jh, jr = divmod(j, 8)
                    gs = max(qt0, 128 * j)
                    w = qt0 + 512 - gs
                    # both heads' scores share one 2-bank psum tile (head1 in
                    # bank 1 at offset 512) so one exp can cover both heads
                    sc = pssc.tile([128, 1024], F32, tag="sc")
                    for head in range(2):
                        nc.tensor.matmul(
                            sc[:, ds(head * 512, w)],
                            qkt[4 + hp][jh][ds(64 * head, 64),
                                            ds(128 * jr, 128)],
                            qkt[hp][qh][ds(64 * head, 64), ds(gs - q0, w)],
                            start=True, stop=True)
                    pj = ppool.tile([128, 1024], BF16, tag="pj")
                    if w == 512:
                        nc.scalar.activation(pj[:], sc[:], EXP, scale=SCALE)
                    else:
                        for head in range(2):
                            nc.scalar.activation(
                                pj[:, ds(head * 512, w)],
                                sc[:, ds(head * 512, w)], EXP, scale=SCALE)
                    if gs == 128 * j:
                        # diagonal block: causal-mask first 128 cols per head
                        for head in range(2):
                            nc.vector.tensor_mul(
                                pj[:, ds(head * 512, 128)],
                                pj[:, ds(head * 512, 128)], trimask[:])
                    for head in range(2):
                        nc.tensor.matmul(
                            pavs[head][:, ds(gs - qt0, w)],
                            vsb[jh][:, jr, 2 * hp + head, :],
                            pj[:, ds(head * 512, w)],
                            start=(j == 0), stop=(j == jmax))
                # copy unnormalized out + l off PSUM so the banks free early
                for head in range(2):
                    nc.vector.tensor_copy(
                        avsb[2 * hp + head][:, ds(qt * 512, 512)],
                        pavs[head][:])
            # gather this pair's softmax denominators early so the final
            # normalize has no DMA chain left to wait on
            for head in range(2):
                h = 2 * hp + head
                nc.sync.dma_start(lsb[ds(h, 1), :], avsb[h][ds(DH, 1), :])

        def normalize(qh, avsb, lsb):
            # 1/l for all 8 heads at once via ACT: exp(-ln(l));
            # Ln and Exp share one table set -> no table switches
            lnl = lpool.tile([8, 1024], F32, tag="lnl")
            nc.scalar.activation(lnl[:], lsb[:], LN)
            rinv = lpool.tile([8, 1024], BF16, tag="rinv")
            nc.scalar.activation(rinv[:], lnl[:], EXP, scale=-1.0)
            for hp in range(4):
                # broadcast 1/l across 64 partitions with a tiny PE matmul:
                # rb[64, 1024] = ssel[:, h, :]^T @ rinv
                rbs = []
                for head in range(2):
                    rb = pssc.tile([64, 1024], F32, tag="sc",
                                   name=f"rb{qh}{hp}{head}")
                    for piece in range(2):
                        nc.tensor.matmul(
                            rb[:, ds(piece * 512, 512)],
                            ssel[:, 2 * hp + head, :],
                            rinv[:, ds(piece * 512, 512)],
                            start=True, stop=True)
                    rbs.append(rb)
                nc.vector.tensor_mul(
                    ot[hp][qh][ds(0, 64), :],
                    avsb[2 * hp][ds(0, DH), :], rbs[0][:])
                ott = lpool.tile([64, 1024], BF16, tag="ott")
                nc.vector.tensor_mul(
                    ott[:], avsb[2 * hp + 1][ds(0, DH), :], rbs[1][:])
                nc.sync.dma_start(ot[hp][qh][ds(64, 64), :], ott[:])

        def out_proj(tt):
            qh, tsub = divmod(tt, 8)
            for half in range(2):
                ps = ps512.tile([128, 512], F32, tag="p512")
                for k in range(4):
                    nc.tensor.matmul(
                        ps[:], ot[k][qh][:, ts(tsub, 128)],
                        wo_sb[k][:, ts(half, 512)],
                        start=(k == 0), stop=(k == 3))
                ysb = ypool.tile([128, 512], F32, tag="ysb")
                nc.vector.tensor_copy(ysb[:], ps[:])
                nc.sync.dma_start(y[ts(tt, 128), ds(half * 512, 512)], ysb[:])

        # ---------------- schedule ----------------------------------------
        avsb = [apool.tile([DH + 1, 1024], BF16, tag=f"av{h}", name=f"av{h}")
                for h in range(8)]
        lsb = lpool.tile([8, 1024], BF16, tag="lsb", name="lsb0")
        for tt in range(8):           # v for token half 0
            v_proj(tt)
        for m in range(4):            # q/k half 0 + attention qh=0
            qk_proj(m, 0)
            qk_proj(4 + m, 0)
            attention(m, 0, avsb, lsb)
        normalize(0, avsb, lsb)
        for tt in range(8, 16):       # v for token half 1
            v_proj(tt)
        avsb = [apool.tile([DH + 1, 1024], BF16, tag=f"av{h}", name=f"avb{h}")
                for h in range(8)]
        lsb = lpool.tile([8, 1024], BF16, tag="lsb", name="lsb1")
        for m in range(4):            # q/k half 1 + attention qh=1
            qk_proj(m, 1)
            qk_proj(4 + m, 1)
            attention(m, 1, avsb, lsb)
            out_proj(2 * m)           # fill PE while ACT works
            out_proj(2 * m + 1)
        normalize(1, avsb, lsb)
        for tt in range(8, 16):
            out_proj(tt)
    return nc


# ---------------- host side ------------------------------------------------

def _rope_tables():
    i = np.arange(DH // 2, dtype=np.float32)
    thetas = np.power(np.float32(10000.0), -2.0 * (i - 1.0) / DH)
    vals = thetas[:, None].astype(np.float32) * \
        np.arange(S, dtype=np.float32)[None, :]
    cos32 = np.cos(vals).astype(np.float32)
    sin32 = np.sin(vals).astype(np.float32)
    CC = np.tile(cos32, (4, 1))
    SSsw = np.concatenate([sin32, -sin32, sin32, -sin32], axis=0)
    return np.ascontiguousarray(CC), np.ascontiguousarray(SSsw)


def _qk_col_perm(g):
    cols = []
    for m in range(4):
        for hh in (2 * m, 2 * m + 1):
            hg = HL * g + hh
            cols += [hg * DH + 2 * i for i in range(32)]
            cols += [hg * DH + 2 * i + 1 for i in range(32)]
    return np.array(cols)


_CACHE = {}


def _get_module():
    if "nc" not in _CACHE:
        nc = bacc.Bacc("TRN2", target_bir_lowering=False, debug=False,
                       num_devices=8)
        build_kernel(nc)
        nc.compile()
        _CACHE["nc"] = nc
    return _CACHE["nc"]


def make_in_maps(x, Wqkv, Wout):
    bf = ml_dtypes.bfloat16
    x = np.asarray(x, np.float32)
    Wqkv = np.asarray(Wqkv, np.float32)
    Wout = np.asarray(Wout, np.float32)
    CC, SSsw = _rope_tables()
    shard = {}
    for g in range(2):
        perm = _qk_col_perm(g)
        vcols = np.arange(HL * g * DH, HL * (g + 1) * DH)
        shard[g] = dict(
            wq=np.ascontiguousarray(Wqkv[:, 0 * INNER:1 * INNER][:, perm].astype(bf)),
            wk=np.ascontiguousarray(Wqkv[:, 1 * INNER:2 * INNER][:, perm].astype(bf)),
            wv=np.ascontiguousarray(Wqkv[:, 2 * INNER:3 * INNER][:, vcols].astype(bf)),
            wo=np.ascontiguousarray(Wout[vcols, :].astype(bf)),
        )
    selm = np.zeros((8, 8, 64), np.float32)
    for h in range(8):
        selm[h, h, :] = 1.0
    selm = np.ascontiguousarray(selm.reshape(8, 512).astype(bf))
    tmask = np.ascontiguousarray(
        np.triu(np.ones((128, 128), np.float32)).astype(bf))
    in_maps = []
    for c in range(8):
        b, g = c // 2, c % 2
        in_maps.append(dict(
            xT=np.ascontiguousarray(x[b].T.astype(bf)), cc=CC, ssw=SSsw,
            selm=selm, tmask=tmask, **shard[g]))
    return in_maps


def kernel(x, Wqkv, Wout, bout):
    bout = np.asarray(bout, np.float32)
    nc = _get_module()
    in_maps = make_in_maps(x, Wqkv, Wout)
    res = run_bass_kernel_spmd(nc, in_maps, core_ids=list(range(8)))
    ys = [r["y"] for r in res.results]
    out = np.stack([ys[2 * b] + ys[2 * b + 1] + bout for b in range(B)])
    return out.astype(np.float32)
